# revision 20
# baseline (speedup 1.0000x reference)
"""Distributed 2-layer GCN on 8 TRN2 NeuronCores (Bass/Tile).

Reference computation (PyG-style GCNConv, f32):
    e  = embed_table[node_tokens]            # [N, 256]
    x0 = e @ Wn^T + bn                       # [N, 128]
    h1 = Ahat @ (x0 @ w1^T) + b1 ; z1 = relu(h1)
    h2 = Ahat @ (z1 @ w2^T) + b2             # output [N, 128]
  with Ahat = D^-1/2 (A + I) D^-1/2, deg from dst(+self loops).
  (Ahat x) @ w^T == Ahat (x @ w^T), so we aggregate first and project after.

Sharding: nodes are partitioned contiguously across the 8 cores (6250 each,
padded to 6272 = 49 tiles of 128). Each core computes x0 for its own nodes
(embedding dma_gather + projection), all-gathers the full feature matrix
between layers, aggregates the edges pointing at its own nodes, projects,
and writes its output shard.

Aggregation design (per layer, per core):
  - Features between layers are stored bf16 [50176, 128] (256B rows, halves
    both the gather traffic and the AllGather payload; PSUM accumulation
    stays f32). dma_gather's int16 indices cover only 32k rows, so gathers
    address the lo rows [0, 25088) and hi rows [25088, 50176) separately.
  - Edges (+ self loops) are bucketed per (dst tile, src half); each bucket
    is padded to whole 128-slot chunks, counts maxed over the 8 cores so all
    cores run one SPMD program.
  - dma_gather fetches up to 8 chunks per instruction, rotating over the 4
    SWDGE queues. Q7 descriptor generation (~4ns/slot, serial) is the
    critical resource, so slot count is kept minimal.
  - Host-precomputed R matrices [128 slots, 128 dsts] bf16 (edge norm at the
    slot's dst column) turn segment-sum into ONE matmul per chunk:
    psum_aggT += msgs_chunk^T @ R_chunk. No per-edge vector ops at all.
  - Tiles are processed in groups of 3 (PSUM bank budget): group-lo chunks,
    then group-hi chunks; each tile's psum accumulates across both blocks.
  - Per dst tile: aggT -> (scalar engine copy) -> w^T matmul -> bias(+relu)
    -> TensorE transpose -> staging -> one DMA per group.
"""

import os

import numpy as np

import concourse.bacc as bacc
import concourse.mybir as mybir
import concourse.tile as tile
from concourse.bass_utils import run_bass_kernel_spmd
from concourse.library_config import mlp

# Problem shape (hardcoded per harness contract)
N = 50000
E = 600000
V = 50000
D_IN = 256
D = 128
NCORES = 8

NPC = N // NCORES            # 6250 nodes per core
TPC = (NPC + 127) // 128     # 49 tiles per core
NPAD = TPC * 128             # 6272 padded nodes per core
NTOT = NCORES * NPAD         # 50176 rows in the all-gathered feature matrix
HALF = NTOT // 2             # 25088 (int16-addressable half)
VLO = V // 2                 # 25000: embedding-table split
EGT = 7                      # tiles per embedding gather group
ENG = TPC // EGT             # embedding groups
GRP = 3                      # dst tiles per aggregation group (PSUM budget)
GMAXC = 8                    # max chunks (x128 slots) per dma_gather
NQ = 4                       # SWDGE queues
F32 = mybir.dt.float32
BF16 = mybir.dt.bfloat16
I16 = mybir.dt.int16
STAGE = int(os.environ.get("KSTAGE", "4"))


def _wrap_idx(idx_linear):
    """[n] -> [128, n/16] int16: position j at [j%16, j//16], replicated x8."""
    n = idx_linear.shape[0]
    assert n % 16 == 0
    w = idx_linear.astype(np.int16).reshape(-1, 16).T
    return np.tile(w, (8, 1))


def _groups():
    gs = []
    t = 0
    while t < TPC:
        gs.append(list(range(t, min(t + GRP, TPC))))
        t += GRP
    return gs


def _preprocess(node_tokens, edge_index):
    """Build per-core host arrays + the (core-uniform) chunk schedule."""
    src = np.asarray(edge_index[0], dtype=np.int64)
    dst = np.asarray(edge_index[1], dtype=np.int64)
    tok = np.asarray(node_tokens, dtype=np.int64)

    deg = (np.bincount(dst, minlength=N) + 1).astype(np.float32)
    dinv = (1.0 / np.sqrt(deg)).astype(np.float32)

    a_src = src
    a_dst = dst
    a_norm = (dinv[src] * dinv[dst]).astype(np.float32)

    core = a_dst // NPC
    dloc = a_dst % NPC
    tloc = dloc // 128
    dcol = (dloc % 128).astype(np.int64)
    src_gid = (a_src // NPC) * NPAD + (a_src % NPC)
    half = (src_gid >= HALF).astype(np.int64)
    idx16 = np.where(half == 0, src_gid, src_gid - HALF)

    wh = (dcol >= 64).astype(np.int64)   # 64-dst window within tile
    key = ((core * TPC + tloc) * 2 + half) * 2 + wh
    order = np.argsort(key, kind="stable")
    idx16_s = idx16[order]
    dcol_s = dcol[order]
    norm_s = a_norm[order]
    NB = NCORES * TPC * 4
    counts_raw = np.bincount(key[order], minlength=NB).reshape(NCORES, TPC, 2, 2)
    starts = np.zeros(NB + 1, dtype=np.int64)
    np.cumsum(counts_raw.reshape(-1), out=starts[1:])

    # dedup srcs within each bucket, then chunk counts = max over cores
    dedup = []   # per (c,t,h,w): (uidx, dcols, norms) with uslot per edge
    counts = np.zeros((NCORES, TPC, 2, 2), np.int64)
    for c in range(NCORES):
        for t in range(TPC):
            for h in (0, 1):
                for w in (0, 1):
                    k = ((c * TPC + t) * 2 + h) * 2 + w
                    s0, ne = starts[k], int(counts_raw[c, t, h, w])
                    uidx, inv = np.unique(idx16_s[s0 : s0 + ne],
                                          return_inverse=True)
                    dedup.append((uidx, inv, dcol_s[s0 : s0 + ne] - w * 64,
                                  norm_s[s0 : s0 + ne]))
                    counts[c, t, h, w] = uidx.shape[0]
    cnt = np.maximum(1, -(-counts.max(axis=0) // 128))  # [TPC, 2, 2]

    # linear chunk order: per group: lo chunks of its tiles, then hi chunks.
    # sched: per chunk (tile, first-of-tile, last-of-tile)
    # gathers: list of (chunk_off, n_chunks, half)
    sched = []           # per chunk: (tile, win, open_flag, stop_flag)
    gathers = []
    chunk_of = {}        # (t, h, w) -> first linear chunk index
    for g in _groups():
        for h in (0, 1):
            blk0 = len(sched)
            for t in g:
                for w in (0, 1):
                    chunk_of[(t, h, w)] = len(sched)
                    k = int(cnt[t, h, w])
                    for i in range(k):
                        sched.append((t, w, h == 0 and w == 0 and i == 0,
                                      h == 1 and w == 1 and i == k - 1))
            nblk = len(sched) - blk0
            off = blk0
            while nblk > 0:
                n = min(GMAXC, nblk)
                gathers.append((off, n, h))
                off += n
                nblk -= n
    tot_chunks = len(sched)

    import ml_dtypes
    per_core = []
    for c in range(NCORES):
        idx_lin = np.zeros(tot_chunks * 128, np.int64)
        rflat = np.zeros((tot_chunks * 128, 64), np.float32)
        for t in range(TPC):
            for h in (0, 1):
                for w in (0, 1):
                    b = ((c * TPC + t) * 2 + h) * 2 + w
                    uidx, inv, lcols, nms = dedup[b]
                    nu = uidx.shape[0]
                    base = chunk_of[(t, h, w)] * 128
                    idx_lin[base : base + nu] = uidx
                    np.add.at(rflat, (base + inv, lcols), nms)

        blocks = []
        for off, n, _h in gathers:
            blocks.append(_wrap_idx(idx_lin[off * 128 : (off + n) * 128]))
        gidx = np.concatenate(blocks, axis=1)

        # rmeta [128 slots, chunk*64 dst] bf16
        rmeta = np.ascontiguousarray(
            rflat.reshape(tot_chunks, 128, 64).transpose(1, 0, 2)
            .reshape(128, tot_chunks * 64)).astype(np.float32)
        import ml_dtypes
        rmeta = rmeta.astype(ml_dtypes.bfloat16)

        # embedding gather indices (per 7-tile group, lo then hi)
        tc_ = tok[c * NPC : (c + 1) * NPC]
        tpad = np.concatenate([tc_, np.zeros(NPAD - NPC, np.int64)])
        lo = np.where(tpad < VLO, tpad, VLO)       # VLO = appended zero row
        hi = np.where(tpad >= VLO, tpad - VLO, V - VLO)
        eblocks = []
        for g in range(ENG):
            sl = slice(g * EGT * 128, (g + 1) * EGT * 128)
            eblocks.append(_wrap_idx(lo[sl]))
            eblocks.append(_wrap_idx(hi[sl]))
        eidx = np.concatenate(eblocks, axis=1)

        rself = np.zeros((128, TPC * 128), np.float32)
        nodes = np.arange(NPC)
        dv2 = dinv[c * NPC : (c + 1) * NPC] ** 2
        rself[nodes % 128, (nodes // 128) * 128 + nodes % 128] = dv2
        rself = rself.astype(ml_dtypes.bfloat16)

        per_core.append({"gidx": gidx, "rmeta": rmeta, "eidx": eidx,
                         "rself": rself})

    layout = {"sched": sched, "gathers": gathers, "tot_chunks": tot_chunks}
    return per_core, layout


def _build(layout):
    sched = layout["sched"]
    gathers = layout["gathers"]
    tot_chunks = layout["tot_chunks"]
    GCOLS = tot_chunks * 8
    ECOLS = ENG * 2 * EGT * 8

    nc = bacc.Bacc("TRN2", target_bir_lowering=False, debug=False,
                   num_devices=NCORES, num_swdge_queues=NQ)

    tab_lo = nc.dram_tensor("tab_lo", [VLO + 1, D_IN], BF16, kind="ExternalInput")
    tab_hi = nc.dram_tensor("tab_hi", [V - VLO + 1, D_IN], BF16, kind="ExternalInput")
    eidx_d = nc.dram_tensor("eidx", [128, ECOLS], I16, kind="ExternalInput")
    gidx_d = nc.dram_tensor("gidx", [128, GCOLS], I16, kind="ExternalInput")
    rmeta_d = nc.dram_tensor("rmeta", [128, tot_chunks * 64], BF16,
                             kind="ExternalInput")
    rself_d = nc.dram_tensor("rself", [128, TPC * 128], BF16,
                             kind="ExternalInput")
    wn_d = nc.dram_tensor("wn", [128, 2, D], BF16, kind="ExternalInput")
    w1t_d = nc.dram_tensor("w1t", [128, D], BF16, kind="ExternalInput")
    w2t_d = nc.dram_tensor("w2t", [128, D], BF16, kind="ExternalInput")
    bias_d = nc.dram_tensor("bias", [128, 3], F32, kind="ExternalInput")
    ident_d = nc.dram_tensor("ident", [128, 128], F32, kind="ExternalInput")
    identb_d = nc.dram_tensor("identb", [128, 128], BF16, kind="ExternalInput")
    out_d = nc.dram_tensor("out", [NPAD, D], F32, kind="ExternalOutput")

    ACT = mybir.ActivationFunctionType

    with tile.TileContext(nc) as tc:
        with (
            tc.tile_pool(name="const", bufs=1) as cp,
            tc.tile_pool(name="embg", bufs=2) as embg,
            tc.tile_pool(name="msgs", bufs=8) as msgp,
            tc.tile_pool(name="rmat", bufs=8) as rmp,
            tc.tile_pool(name="work", bufs=3) as wk,
            tc.tile_pool(name="stage", bufs=2) as stg,
            tc.tile_pool(name="psA", bufs=4, space="PSUM") as psA,
            tc.tile_pool(name="psB", bufs=2, space="PSUM") as psB,
            tc.tile_pool(name="psC", bufs=2, space="PSUM") as psC,
            tc.tile_pool(name="dram", bufs=1, space="DRAM") as dram,
        ):
            nc.gpsimd.load_library(mlp)

            eidx_sb = cp.tile([128, ECOLS], I16)
            gidx_sb = cp.tile([128, GCOLS], I16)
            wn_sb = cp.tile([128, 2, D], BF16)
            w1t_sb = cp.tile([128, D], BF16)
            w2t_sb = cp.tile([128, D], BF16)
            bias_sb = cp.tile([128, 3], F32)
            ident_sb = cp.tile([128, 128], F32)
            identb_sb = cp.tile([128, 128], BF16)
            nc.sync.dma_start(eidx_sb[:], eidx_d[:])
            nc.sync.dma_start(gidx_sb[:], gidx_d[:])
            nc.sync.dma_start(wn_sb[:], wn_d[:])
            nc.sync.dma_start(w1t_sb[:], w1t_d[:])
            nc.sync.dma_start(w2t_sb[:], w2t_d[:])
            nc.sync.dma_start(bias_sb[:], bias_d[:])
            nc.sync.dma_start(ident_sb[:], ident_d[:])
            nc.sync.dma_start(identb_sb[:], identb_d[:])

            z0_loc = dram.tile([NPAD, D], BF16)
            z1_loc = dram.tile([NPAD, D], BF16)
            z0_full = dram.tile([NTOT, D], BF16, addr_space="Shared")
            z1_full = dram.tile([NTOT, D], BF16, addr_space="Shared")

            qn = [0]

            def next_q():
                qn[0] = (qn[0] + 1) % NQ
                return qn[0]

            # ---- embedding + input projection: z0 = tab[tok] @ Wn^T + bn ----
            for g in range(ENG):
                nidx = EGT * 128
                e_lo = embg.tile([128, EGT, D_IN], BF16, name="e_lo", tag="e_lo")
                e_hi = embg.tile([128, EGT, D_IN], BF16, name="e_hi", tag="e_hi")
                off = g * 2 * EGT * 8
                nc.gpsimd.dma_gather(e_lo[:], tab_lo[:],
                                     eidx_sb[:, off : off + EGT * 8],
                                     nidx, nidx, D_IN, queue_num=next_q())
                nc.gpsimd.dma_gather(e_hi[:], tab_hi[:],
                                     eidx_sb[:, off + EGT * 8 : off + 2 * EGT * 8],
                                     nidx, nidx, D_IN, queue_num=next_q())
                staging = stg.tile([128, EGT, D], BF16, name="stage0", tag="st0")
                for tt in range(EGT):
                    x0T_ps = psB.tile([128, 128], F32, name="x0T", tag="pB")
                    e_sb = wk.tile([128, D_IN], BF16, name="e_sb", tag="e_sb")
                    nc.vector.tensor_tensor(e_sb[:], e_lo[:, tt, :],
                                            e_hi[:, tt, :], mybir.AluOpType.add)
                    for kc in range(2):
                        eT_ps = psA.tile([128, 128], BF16, name="eT", tag="pA")
                        nc.tensor.matmul(
                            eT_ps[:], e_sb[:, kc * 128 : (kc + 1) * 128],
                            identb_sb[:], is_transpose=True, start=True, stop=True)
                        eT_sb = wk.tile([128, 128], BF16, name="eT_sb", tag="eT_sb")
                        nc.scalar.activation(eT_sb[:], eT_ps[:], ACT.Copy)
                        nc.tensor.matmul(x0T_ps[:], wn_sb[:, kc, :], eT_sb[:],
                                         start=(kc == 0), stop=(kc == 1))
                    x0T_sb = wk.tile([128, 128], BF16, name="x0T_sb", tag="x0T_sb")
                    nc.scalar.activation(x0T_sb[:], x0T_ps[:], ACT.Identity,
                                         bias=bias_sb[:, 0:1])
                    x0_ps = psC.tile([128, 128], BF16, name="x0", tag="pC")
                    nc.tensor.matmul(x0_ps[:], x0T_sb[:], identb_sb[:],
                                     is_transpose=True, start=True, stop=True)
                    nc.scalar.activation(staging[:, tt, :], x0_ps[:], ACT.Copy)
                dst_rows = z0_loc[g * EGT * 128 : (g + 1) * EGT * 128, :]
                nc.sync.dma_start(
                    dst_rows.rearrange("(t p) f -> p t f", p=128), staging[:])

            if STAGE >= 2:
                nc.gpsimd.collective_compute(
                    "AllGather", mybir.AluOpType.bypass,
                    replica_groups=[list(range(NCORES))],
                    ins=[z0_loc.opt()], outs=[z0_full.opt()])

            # ---- GCN layers ----
            def gcn_layer(z_full, z_loc, wt_sb, bias_col, relu, dest,
                          dest_f32):
                open_ps = {}
                staging = [None]
                grp_sb = {}
                views = (z_full[0:HALF, :], z_full[HALF:NTOT, :])

                def open_tile(t):
                    open_ps[t] = psA.tile([128, 128], F32, name="aggT", tag="pA")
                    if t % GRP == 0:
                        ntile = min(GRP, TPC - t)
                        staging[0] = stg.tile(
                            [128, ntile, D], F32 if dest_f32 else BF16,
                            name="stage1", tag="st1")
                        zs = stg.tile([128, ntile, D], BF16, name="zself",
                                      tag="zself")
                        rs = stg.tile([128, ntile, 128], BF16, name="rself",
                                      tag="rself")
                        nc.sync.dma_start(
                            zs[:], z_loc[t * 128 : (t + ntile) * 128, :]
                            .rearrange("(t p) f -> p t f", p=128))
                        nc.sync.dma_start(
                            rs[:], rself_d[:, t * 128 : (t + ntile) * 128]
                            .rearrange("p (t f) -> p t f", f=128))
                        grp_sb["zs"], grp_sb["rs"] = zs, rs
                    # self-loop contribution opens the accumulation group
                    nc.tensor.matmul(open_ps[t][:], grp_sb["zs"][:, t % GRP, :],
                                     grp_sb["rs"][:, t % GRP, :],
                                     start=True, stop=False)

                def close_tile(t):
                    aggT_sb = wk.tile([128, 128], BF16, name="aggT_sb",
                                      tag="aggT_sb")
                    nc.scalar.activation(aggT_sb[:], open_ps.pop(t)[:], ACT.Copy)
                    yT_ps = psB.tile([128, 128], F32, name="yT", tag="pB")
                    nc.tensor.matmul(yT_ps[:], wt_sb[:], aggT_sb[:],
                                     start=True, stop=True)
                    yT_sb = wk.tile([128, 128], BF16, name="yT_sb", tag="yT_sb")
                    nc.scalar.activation(yT_sb[:], yT_ps[:],
                                         ACT.Relu if relu else ACT.Identity,
                                         bias=bias_col)
                    y_ps = psC.tile([128, 128], BF16, name="y", tag="pC")
                    nc.tensor.matmul(y_ps[:], yT_sb[:], identb_sb[:],
                                     is_transpose=True, start=True, stop=True)
                    nc.scalar.activation(staging[0][:, t % GRP, :], y_ps[:],
                                         ACT.Copy)
                    if t % GRP == GRP - 1 or t == TPC - 1:
                        g0 = (t // GRP) * GRP
                        ntile = t - g0 + 1
                        dst_rows = dest[g0 * 128 : (g0 + ntile) * 128, :]
                        nc.sync.dma_start(
                            dst_rows.rearrange("(t p) f -> p t f", p=128),
                            staging[0][:])

                for off, n, h in gathers:
                    msgs = msgp.tile([128, GMAXC, D], BF16, name="m", tag="m")
                    nc.gpsimd.dma_gather(
                        msgs[:, 0:n, :], views[h],
                        gidx_sb[:, off * 8 : (off + n) * 8],
                        n * 128, n * 128, D, queue_num=next_q())
                    rmat = rmp.tile([128, GMAXC, 64], BF16, name="r", tag="r")
                    nc.sync.dma_start(
                        rmat[:, 0:n, :],
                        rmeta_d[:, off * 64 : (off + n) * 64]
                        .rearrange("p (c f) -> p c f", f=64))
                    for k in range(n):
                        t, w, first, last = sched[off + k]
                        if first:
                            open_tile(t)
                        nc.tensor.matmul(
                            open_ps[t][:, w * 64 : w * 64 + 64],
                            msgs[:, k, :], rmat[:, k, :],
                            start=False, stop=last)
                        if last:
                            close_tile(t)

            if STAGE >= 3:
                gcn_layer(z0_full, z0_loc, w1t_sb, bias_sb[:, 1:2], True, z1_loc, False)
            if STAGE == 3:
                nc.gpsimd.dma_start(out_d[:], z1_loc[:])
            if STAGE >= 4:
                nc.gpsimd.collective_compute(
                    "AllGather", mybir.AluOpType.bypass,
                    replica_groups=[list(range(NCORES))],
                    ins=[z1_loc.opt()], outs=[z1_full.opt()])
                gcn_layer(z1_full, z1_loc, w2t_sb, bias_sb[:, 2:3], False,
                          out_d.ap(), True)

    nc.compile()
    return nc


_CACHE = {}


def _run(inputs, trace=False):
    import ml_dtypes

    node_tokens = np.asarray(inputs["node_tokens"])
    edge_index = np.asarray(inputs["edge_index"])
    embed_table = np.asarray(inputs["embed_table"], dtype=np.float32)
    Wn = np.asarray(inputs["W_node_w"], dtype=np.float32)
    bn = np.asarray(inputs["W_node_b"], dtype=np.float32)
    w1 = np.asarray(inputs["w1"], dtype=np.float32)
    b1 = np.asarray(inputs["b1"], dtype=np.float32)
    w2 = np.asarray(inputs["w2"], dtype=np.float32)
    b2 = np.asarray(inputs["b2"], dtype=np.float32)

    per_core, layout = _preprocess(node_tokens, edge_index)

    if "nc" not in _CACHE:
        _CACHE["nc"] = _build(layout)
    nc = _CACHE["nc"]

    tab_lo = np.concatenate([embed_table[:VLO], np.zeros((1, D_IN), np.float32)]
                            ).astype(ml_dtypes.bfloat16)
    tab_hi = np.concatenate([embed_table[VLO:], np.zeros((1, D_IN), np.float32)]
                            ).astype(ml_dtypes.bfloat16)
    WnT = Wn.T.copy()
    wn = WnT.reshape(2, 128, D).transpose(1, 0, 2).astype(ml_dtypes.bfloat16)
    bias = np.stack([bn, b1, b2], axis=1).astype(np.float32)
    ident = np.eye(128, dtype=np.float32)
    identb = np.eye(128, dtype=ml_dtypes.bfloat16)

    in_maps = []
    for c in range(NCORES):
        in_maps.append({
            "tab_lo": tab_lo, "tab_hi": tab_hi,
            "eidx": per_core[c]["eidx"],
            "gidx": per_core[c]["gidx"],
            "rmeta": per_core[c]["rmeta"],
            "rself": per_core[c]["rself"],
            "wn": wn,
            "w1t": w1.T.astype(ml_dtypes.bfloat16),
            "w2t": w2.T.astype(ml_dtypes.bfloat16),
            "bias": bias, "ident": ident, "identb": identb,
        })

    res = run_bass_kernel_spmd(nc, in_maps, core_ids=list(range(NCORES)),
                               trace=trace)
    out = np.concatenate([res.results[c]["out"][:NPC] for c in range(NCORES)],
                         axis=0)
    return out.astype(np.float32), res


def kernel(**inputs):
    out, _ = _run(inputs, trace=False)
    return out


# revision 21
# speedup vs baseline: 1.3570x; 1.3570x over previous
"""Distributed 2-layer GCN on 8 TRN2 NeuronCores (Bass/Tile).

Reference computation (PyG-style GCNConv, f32):
    e  = embed_table[node_tokens]            # [N, 256]
    x0 = e @ Wn^T + bn                       # [N, 128]
    h1 = Ahat @ (x0 @ w1^T) + b1 ; z1 = relu(h1)
    h2 = Ahat @ (z1 @ w2^T) + b2             # output [N, 128]
  with Ahat = D^-1/2 (A + I) D^-1/2, deg from dst(+self loops).
  (Ahat x) @ w^T == Ahat (x @ w^T), so we aggregate first and project after.

Sharding: nodes are partitioned contiguously across the 8 cores (6250 each,
padded to 6272 = 49 tiles of 128). Each core computes x0 for its own nodes
(embedding dma_gather + projection), all-gathers the full feature matrix
between layers, aggregates the edges pointing at its own nodes, projects,
and writes its output shard.

Aggregation design (per layer, per core):
  - Features between layers are stored bf16 [50176, 128] (256B rows, halves
    both the gather traffic and the AllGather payload; PSUM accumulation
    stays f32). dma_gather's int16 indices cover only 32k rows, so gathers
    address the lo rows [0, 25088) and hi rows [25088, 50176) separately.
  - Edges (+ self loops) are bucketed per (dst tile, src half); each bucket
    is padded to whole 128-slot chunks, counts maxed over the 8 cores so all
    cores run one SPMD program.
  - dma_gather fetches up to 8 chunks per instruction, rotating over the 4
    SWDGE queues. Q7 descriptor generation (~4ns/slot, serial) is the
    critical resource, so slot count is kept minimal.
  - Host-precomputed R matrices [128 slots, 128 dsts] bf16 (edge norm at the
    slot's dst column) turn segment-sum into ONE matmul per chunk:
    psum_aggT += msgs_chunk^T @ R_chunk. No per-edge vector ops at all.
  - Tiles are processed in groups of 3 (PSUM bank budget): group-lo chunks,
    then group-hi chunks; each tile's psum accumulates across both blocks.
  - Per dst tile: aggT -> (scalar engine copy) -> w^T matmul -> bias(+relu)
    -> TensorE transpose -> staging -> one DMA per group.
"""

import os

import numpy as np

import concourse.bacc as bacc
import concourse.mybir as mybir
import concourse.tile as tile
from concourse.bass_utils import run_bass_kernel_spmd
from concourse.library_config import mlp

# Problem shape (hardcoded per harness contract)
N = 50000
E = 600000
V = 50000
D_IN = 256
D = 128
NCORES = 8

NPC = N // NCORES            # 6250 nodes per core
TPC = (NPC + 127) // 128     # 49 tiles per core
NPAD = TPC * 128             # 6272 padded nodes per core
NTOT = NCORES * NPAD         # 50176 rows in the all-gathered feature matrix
HALF = NTOT // 2             # 25088 (int16-addressable half)
VLO = V // 2                 # 25000: embedding-table split
EGT = 7                      # tiles per embedding gather group
ENG = TPC // EGT             # embedding groups
GRP = 3                      # dst tiles per aggregation group (PSUM budget)
GMAXC = 8                    # max chunks (x128 slots) per dma_gather
NQ = 4                       # SWDGE queues
F32 = mybir.dt.float32
BF16 = mybir.dt.bfloat16
I16 = mybir.dt.int16
STAGE = int(os.environ.get("KSTAGE", "4"))


def _wrap_idx(idx_linear):
    """[n] -> [128, n/16] int16: position j at [j%16, j//16], replicated x8."""
    n = idx_linear.shape[0]
    assert n % 16 == 0
    w = idx_linear.astype(np.int16).reshape(-1, 16).T
    return np.tile(w, (8, 1))


def _groups():
    gs = []
    t = 0
    while t < TPC:
        gs.append(list(range(t, min(t + GRP, TPC))))
        t += GRP
    return gs


def _preprocess(node_tokens, edge_index):
    """Build per-core host arrays + the (core-uniform) chunk schedule."""
    src = np.asarray(edge_index[0], dtype=np.int64)
    dst = np.asarray(edge_index[1], dtype=np.int64)
    tok = np.asarray(node_tokens, dtype=np.int64)

    deg = (np.bincount(dst, minlength=N) + 1).astype(np.float32)
    dinv = (1.0 / np.sqrt(deg)).astype(np.float32)

    a_src = src
    a_dst = dst
    a_norm = (dinv[src] * dinv[dst]).astype(np.float32)

    core = a_dst // NPC
    dloc = a_dst % NPC
    tloc = dloc // 128
    dcol = (dloc % 128).astype(np.int64)
    src_gid = (a_src // NPC) * NPAD + (a_src % NPC)
    half = (src_gid >= HALF).astype(np.int64)
    idx16 = np.where(half == 0, src_gid, src_gid - HALF)

    key = (core * TPC + tloc) * 2 + half
    order = np.argsort(key, kind="stable")
    idx16_s = idx16[order]
    dcol_s = dcol[order]
    norm_s = a_norm[order]
    counts_raw = np.bincount(key[order], minlength=NCORES * TPC * 2).reshape(
        NCORES, TPC, 2)
    starts = np.zeros(NCORES * TPC * 2 + 1, dtype=np.int64)
    np.cumsum(counts_raw.reshape(-1), out=starts[1:])

    # dedup repeated srcs within each (tile, half) bucket (R rows can carry
    # multiple dst columns), then chunk counts = max over cores
    dedup = {}
    counts = np.zeros((NCORES, TPC, 2), np.int64)
    for c in range(NCORES):
        for t in range(TPC):
            for h in (0, 1):
                k = (c * TPC + t) * 2 + h
                s0, ne = starts[k], int(counts_raw[c, t, h])
                uidx, inv = np.unique(idx16_s[s0 : s0 + ne],
                                      return_inverse=True)
                dedup[(c, t, h)] = (uidx, inv, dcol_s[s0 : s0 + ne],
                                    norm_s[s0 : s0 + ne])
                counts[c, t, h] = uidx.shape[0]

    # chunks per (tile, half): max over cores
    cnt = np.maximum(1, -(-counts.max(axis=0) // 128))  # [TPC, 2]

    # linear chunk order: per group: lo chunks of its tiles, then hi chunks.
    # sched: per chunk (tile, first-of-tile, last-of-tile)
    # gathers: list of (chunk_off, n_chunks, half)
    sched = []
    gathers = []
    chunk_of = {}       # (t, h) -> first linear chunk index
    for g in _groups():
        for h in (0, 1):
            blk0 = len(sched)
            for t in g:
                chunk_of[(t, h)] = len(sched)
                k = int(cnt[t, h])
                for i in range(k):
                    sched.append((t, h == 0 and i == 0, h == 1 and i == k - 1))
            nblk = len(sched) - blk0
            off = blk0
            while nblk > 0:
                n = min(GMAXC, nblk)
                gathers.append((off, n, h))
                off += n
                nblk -= n
    tot_chunks = len(sched)

    import ml_dtypes
    per_core = []
    for c in range(NCORES):
        idx_lin = np.zeros(tot_chunks * 128, np.int64)
        rflat = np.zeros((tot_chunks * 128, 128), np.float32)
        for t in range(TPC):
            for h in (0, 1):
                uidx, inv, dcols, nms = dedup[(c, t, h)]
                base = chunk_of[(t, h)] * 128
                idx_lin[base : base + uidx.shape[0]] = uidx
                np.add.at(rflat, (base + inv, dcols), nms)

        blocks = []
        for off, n, _h in gathers:
            blocks.append(_wrap_idx(idx_lin[off * 128 : (off + n) * 128]))
        gidx = np.concatenate(blocks, axis=1)

        # rmeta [128 slots, chunk*128 dst] bf16
        rmeta = np.ascontiguousarray(
            rflat.reshape(tot_chunks, 128, 128).transpose(1, 0, 2)
            .reshape(128, tot_chunks * 128)).astype(np.float32)
        import ml_dtypes
        rmeta = rmeta.astype(ml_dtypes.bfloat16)

        # embedding gather indices (per 7-tile group, lo then hi)
        tc_ = tok[c * NPC : (c + 1) * NPC]
        tpad = np.concatenate([tc_, np.zeros(NPAD - NPC, np.int64)])
        lo = np.where(tpad < VLO, tpad, VLO)       # VLO = appended zero row
        hi = np.where(tpad >= VLO, tpad - VLO, V - VLO)
        eblocks = []
        for g in range(ENG):
            sl = slice(g * EGT * 128, (g + 1) * EGT * 128)
            eblocks.append(_wrap_idx(lo[sl]))
            eblocks.append(_wrap_idx(hi[sl]))
        eidx = np.concatenate(eblocks, axis=1)

        rself = np.zeros((128, TPC * 128), np.float32)
        nodes = np.arange(NPC)
        dv2 = dinv[c * NPC : (c + 1) * NPC] ** 2
        rself[nodes % 128, (nodes // 128) * 128 + nodes % 128] = dv2
        rself = rself.astype(ml_dtypes.bfloat16)

        per_core.append({"gidx": gidx, "rmeta": rmeta, "eidx": eidx,
                         "rself": rself})

    layout = {"sched": sched, "gathers": gathers, "tot_chunks": tot_chunks}
    return per_core, layout


def _build(layout):
    sched = layout["sched"]
    gathers = layout["gathers"]
    tot_chunks = layout["tot_chunks"]
    GCOLS = tot_chunks * 8
    ECOLS = ENG * 2 * EGT * 8

    nc = bacc.Bacc("TRN2", target_bir_lowering=False, debug=False,
                   num_devices=NCORES, num_swdge_queues=NQ)

    tab_lo = nc.dram_tensor("tab_lo", [VLO + 1, D_IN], BF16, kind="ExternalInput")
    tab_hi = nc.dram_tensor("tab_hi", [V - VLO + 1, D_IN], BF16, kind="ExternalInput")
    eidx_d = nc.dram_tensor("eidx", [128, ECOLS], I16, kind="ExternalInput")
    gidx_d = nc.dram_tensor("gidx", [128, GCOLS], I16, kind="ExternalInput")
    rmeta_d = nc.dram_tensor("rmeta", [128, tot_chunks * 128], BF16,
                             kind="ExternalInput")
    rself_d = nc.dram_tensor("rself", [128, TPC * 128], BF16,
                             kind="ExternalInput")
    wn_d = nc.dram_tensor("wn", [128, 2, D], BF16, kind="ExternalInput")
    w1t_d = nc.dram_tensor("w1t", [128, D], BF16, kind="ExternalInput")
    w2t_d = nc.dram_tensor("w2t", [128, D], BF16, kind="ExternalInput")
    bias_d = nc.dram_tensor("bias", [128, 3], F32, kind="ExternalInput")
    ident_d = nc.dram_tensor("ident", [128, 128], F32, kind="ExternalInput")
    identb_d = nc.dram_tensor("identb", [128, 128], BF16, kind="ExternalInput")
    out_d = nc.dram_tensor("out", [NPAD, D], F32, kind="ExternalOutput")

    ACT = mybir.ActivationFunctionType

    with tile.TileContext(nc) as tc:
        with (
            tc.tile_pool(name="const", bufs=1) as cp,
            tc.tile_pool(name="embg", bufs=2) as embg,
            tc.tile_pool(name="msgs", bufs=8) as msgp,
            tc.tile_pool(name="rmat", bufs=8) as rmp,
            tc.tile_pool(name="work", bufs=3) as wk,
            tc.tile_pool(name="stage", bufs=2) as stg,
            tc.tile_pool(name="psA", bufs=4, space="PSUM") as psA,
            tc.tile_pool(name="psB", bufs=2, space="PSUM") as psB,
            tc.tile_pool(name="psC", bufs=2, space="PSUM") as psC,
            tc.tile_pool(name="dram", bufs=1, space="DRAM") as dram,
        ):
            nc.gpsimd.load_library(mlp)

            eidx_sb = cp.tile([128, ECOLS], I16)
            gidx_sb = cp.tile([128, GCOLS], I16)
            wn_sb = cp.tile([128, 2, D], BF16)
            w1t_sb = cp.tile([128, D], BF16)
            w2t_sb = cp.tile([128, D], BF16)
            bias_sb = cp.tile([128, 3], F32)
            ident_sb = cp.tile([128, 128], F32)
            identb_sb = cp.tile([128, 128], BF16)
            nc.sync.dma_start(eidx_sb[:], eidx_d[:])
            nc.sync.dma_start(gidx_sb[:], gidx_d[:])
            nc.sync.dma_start(wn_sb[:], wn_d[:])
            nc.sync.dma_start(w1t_sb[:], w1t_d[:])
            nc.sync.dma_start(w2t_sb[:], w2t_d[:])
            nc.sync.dma_start(bias_sb[:], bias_d[:])
            nc.sync.dma_start(ident_sb[:], ident_d[:])
            nc.sync.dma_start(identb_sb[:], identb_d[:])

            z0_loc = dram.tile([NPAD, D], BF16)
            z1_loc = dram.tile([NPAD, D], BF16)
            z0_full = dram.tile([NTOT, D], BF16, addr_space="Shared")
            z1_full = dram.tile([NTOT, D], BF16, addr_space="Shared")

            qn = [0]

            def next_q():
                qn[0] = (qn[0] + 1) % NQ
                return qn[0]

            # ---- embedding + input projection: z0 = tab[tok] @ Wn^T + bn ----
            for g in range(ENG):
                nidx = EGT * 128
                e_lo = embg.tile([128, EGT, D_IN], BF16, name="e_lo", tag="e_lo")
                e_hi = embg.tile([128, EGT, D_IN], BF16, name="e_hi", tag="e_hi")
                off = g * 2 * EGT * 8
                nc.gpsimd.dma_gather(e_lo[:], tab_lo[:],
                                     eidx_sb[:, off : off + EGT * 8],
                                     nidx, nidx, D_IN, queue_num=next_q())
                nc.gpsimd.dma_gather(e_hi[:], tab_hi[:],
                                     eidx_sb[:, off + EGT * 8 : off + 2 * EGT * 8],
                                     nidx, nidx, D_IN, queue_num=next_q())
                staging = stg.tile([128, EGT, D], BF16, name="stage0", tag="st0")
                for tt in range(EGT):
                    x0T_ps = psB.tile([128, 128], F32, name="x0T", tag="pB")
                    e_sb = wk.tile([128, D_IN], BF16, name="e_sb", tag="e_sb")
                    nc.vector.tensor_tensor(e_sb[:], e_lo[:, tt, :],
                                            e_hi[:, tt, :], mybir.AluOpType.add)
                    for kc in range(2):
                        eT_ps = psA.tile([128, 128], BF16, name="eT", tag="pA")
                        nc.tensor.matmul(
                            eT_ps[:], e_sb[:, kc * 128 : (kc + 1) * 128],
                            identb_sb[:], is_transpose=True, start=True, stop=True)
                        eT_sb = wk.tile([128, 128], BF16, name="eT_sb", tag="eT_sb")
                        nc.scalar.activation(eT_sb[:], eT_ps[:], ACT.Copy)
                        nc.tensor.matmul(x0T_ps[:], wn_sb[:, kc, :], eT_sb[:],
                                         start=(kc == 0), stop=(kc == 1))
                    x0T_sb = wk.tile([128, 128], BF16, name="x0T_sb", tag="x0T_sb")
                    nc.scalar.activation(x0T_sb[:], x0T_ps[:], ACT.Identity,
                                         bias=bias_sb[:, 0:1])
                    x0_ps = psC.tile([128, 128], BF16, name="x0", tag="pC")
                    nc.tensor.matmul(x0_ps[:], x0T_sb[:], identb_sb[:],
                                     is_transpose=True, start=True, stop=True)
                    nc.scalar.activation(staging[:, tt, :], x0_ps[:], ACT.Copy)
                dst_rows = z0_loc[g * EGT * 128 : (g + 1) * EGT * 128, :]
                nc.sync.dma_start(
                    dst_rows.rearrange("(t p) f -> p t f", p=128), staging[:])

            if STAGE >= 2:
                nc.gpsimd.collective_compute(
                    "AllGather", mybir.AluOpType.bypass,
                    replica_groups=[list(range(NCORES))],
                    ins=[z0_loc.opt()], outs=[z0_full.opt()])

            # ---- GCN layers ----
            def gcn_layer(z_full, z_loc, wt_sb, bias_col, relu, dest,
                          dest_f32):
                open_ps = {}
                staging = [None]
                grp_sb = {}
                views = (z_full[0:HALF, :], z_full[HALF:NTOT, :])

                def open_tile(t):
                    open_ps[t] = psA.tile([128, 128], F32, name="aggT", tag="pA")
                    if t % GRP == 0:
                        ntile = min(GRP, TPC - t)
                        staging[0] = stg.tile(
                            [128, ntile, D], F32 if dest_f32 else BF16,
                            name="stage1", tag="st1")
                        zs = stg.tile([128, ntile, D], BF16, name="zself",
                                      tag="zself")
                        rs = stg.tile([128, ntile, 128], BF16, name="rself",
                                      tag="rself")
                        nc.sync.dma_start(
                            zs[:], z_loc[t * 128 : (t + ntile) * 128, :]
                            .rearrange("(t p) f -> p t f", p=128))
                        nc.sync.dma_start(
                            rs[:], rself_d[:, t * 128 : (t + ntile) * 128]
                            .rearrange("p (t f) -> p t f", f=128))
                        grp_sb["zs"], grp_sb["rs"] = zs, rs
                    # self-loop contribution opens the accumulation group
                    nc.tensor.matmul(open_ps[t][:], grp_sb["zs"][:, t % GRP, :],
                                     grp_sb["rs"][:, t % GRP, :],
                                     start=True, stop=False)

                def close_tile(t):
                    aggT_sb = wk.tile([128, 128], BF16, name="aggT_sb",
                                      tag="aggT_sb")
                    nc.scalar.activation(aggT_sb[:], open_ps.pop(t)[:], ACT.Copy)
                    yT_ps = psB.tile([128, 128], F32, name="yT", tag="pB")
                    nc.tensor.matmul(yT_ps[:], wt_sb[:], aggT_sb[:],
                                     start=True, stop=True)
                    yT_sb = wk.tile([128, 128], BF16, name="yT_sb", tag="yT_sb")
                    nc.scalar.activation(yT_sb[:], yT_ps[:],
                                         ACT.Relu if relu else ACT.Identity,
                                         bias=bias_col)
                    y_ps = psC.tile([128, 128], BF16, name="y", tag="pC")
                    nc.tensor.matmul(y_ps[:], yT_sb[:], identb_sb[:],
                                     is_transpose=True, start=True, stop=True)
                    nc.scalar.activation(staging[0][:, t % GRP, :], y_ps[:],
                                         ACT.Copy)
                    if t % GRP == GRP - 1 or t == TPC - 1:
                        g0 = (t // GRP) * GRP
                        ntile = t - g0 + 1
                        dst_rows = dest[g0 * 128 : (g0 + ntile) * 128, :]
                        nc.sync.dma_start(
                            dst_rows.rearrange("(t p) f -> p t f", p=128),
                            staging[0][:])

                for off, n, h in gathers:
                    msgs = msgp.tile([128, GMAXC, D], BF16, name="m", tag="m")
                    nc.gpsimd.dma_gather(
                        msgs[:, 0:n, :], views[h],
                        gidx_sb[:, off * 8 : (off + n) * 8],
                        n * 128, n * 128, D, queue_num=next_q())
                    rmat = rmp.tile([128, GMAXC, 128], BF16, name="r", tag="r")
                    nc.sync.dma_start(
                        rmat[:, 0:n, :],
                        rmeta_d[:, off * 128 : (off + n) * 128]
                        .rearrange("p (c f) -> p c f", f=128))
                    for k in range(n):
                        t, first, last = sched[off + k]
                        if first:
                            open_tile(t)
                        nc.tensor.matmul(open_ps[t][:], msgs[:, k, :],
                                         rmat[:, k, :], start=False, stop=last)
                        if last:
                            close_tile(t)

            if STAGE >= 3:
                gcn_layer(z0_full, z0_loc, w1t_sb, bias_sb[:, 1:2], True, z1_loc, False)
            if STAGE == 3:
                nc.gpsimd.dma_start(out_d[:], z1_loc[:])
            if STAGE >= 4:
                nc.gpsimd.collective_compute(
                    "AllGather", mybir.AluOpType.bypass,
                    replica_groups=[list(range(NCORES))],
                    ins=[z1_loc.opt()], outs=[z1_full.opt()])
                gcn_layer(z1_full, z1_loc, w2t_sb, bias_sb[:, 2:3], False,
                          out_d.ap(), True)

    nc.compile()
    return nc


_CACHE = {}


def _run(inputs, trace=False):
    import ml_dtypes

    node_tokens = np.asarray(inputs["node_tokens"])
    edge_index = np.asarray(inputs["edge_index"])
    embed_table = np.asarray(inputs["embed_table"], dtype=np.float32)
    Wn = np.asarray(inputs["W_node_w"], dtype=np.float32)
    bn = np.asarray(inputs["W_node_b"], dtype=np.float32)
    w1 = np.asarray(inputs["w1"], dtype=np.float32)
    b1 = np.asarray(inputs["b1"], dtype=np.float32)
    w2 = np.asarray(inputs["w2"], dtype=np.float32)
    b2 = np.asarray(inputs["b2"], dtype=np.float32)

    per_core, layout = _preprocess(node_tokens, edge_index)

    if "nc" not in _CACHE:
        _CACHE["nc"] = _build(layout)
    nc = _CACHE["nc"]

    tab_lo = np.concatenate([embed_table[:VLO], np.zeros((1, D_IN), np.float32)]
                            ).astype(ml_dtypes.bfloat16)
    tab_hi = np.concatenate([embed_table[VLO:], np.zeros((1, D_IN), np.float32)]
                            ).astype(ml_dtypes.bfloat16)
    WnT = Wn.T.copy()
    wn = WnT.reshape(2, 128, D).transpose(1, 0, 2).astype(ml_dtypes.bfloat16)
    bias = np.stack([bn, b1, b2], axis=1).astype(np.float32)
    ident = np.eye(128, dtype=np.float32)
    identb = np.eye(128, dtype=ml_dtypes.bfloat16)

    in_maps = []
    for c in range(NCORES):
        in_maps.append({
            "tab_lo": tab_lo, "tab_hi": tab_hi,
            "eidx": per_core[c]["eidx"],
            "gidx": per_core[c]["gidx"],
            "rmeta": per_core[c]["rmeta"],
            "rself": per_core[c]["rself"],
            "wn": wn,
            "w1t": w1.T.astype(ml_dtypes.bfloat16),
            "w2t": w2.T.astype(ml_dtypes.bfloat16),
            "bias": bias, "ident": ident, "identb": identb,
        })

    res = run_bass_kernel_spmd(nc, in_maps, core_ids=list(range(NCORES)),
                               trace=trace)
    out = np.concatenate([res.results[c]["out"][:NPC] for c in range(NCORES)],
                         axis=0)
    return out.astype(np.float32), res


def kernel(**inputs):
    out, _ = _run(inputs, trace=False)
    return out
